# revision 12
# baseline (speedup 1.0000x reference)
"""Bass/Tile kernel for nn_BinaryClassifierChain on 8 trn2 cores (v7).

Math (per reference.py):
  wc   = softmax(word_class_features, axis=0)            # over batch dim
  base = concat([features, wc], -1)                      # [B, W, 1088]
  L    = base @ W[:, :1088].T + b                        # [B, W, 32]
  chain: p_i = sigmoid(L_i + sum_{j<i} Wbin[i, j] p_j)   # Wbin = W[:, 1088:]

Sharding: pure data-parallel over the words dim (1024 = 8 x 128); the
batch-softmax stays intact per shard.

v7 vs v6 (v6 trace: DVE oversubscribed in groups 8-15 stretched the
pipeline 25us; FMA-form chain ops cost ~0.5us each so the 4-op FMA step
was no faster than mul+reduce):
  - chain step = mul on GpSimd (was idle), reduce on DVE, sigmoid on
    ACT: three engines, one op each, for every chunk.
  - chunks of 16 batches: A bins over groups 4-11, B over 8-15, C bins
    0-15 over 12-15; tail = C 16-31 + D 0-31 as two interleaved chains.
  - 6 of 16 groups load their features via the gpsimd SWDGE cast path
    (f32->bf16 in-flight): those groups transpose in bf16 (1 PE
    cycle/row vs 2) and evacuate bf16->bf16; HWDGE carries the rest
    concurrently, so aggregate HBM pull approaches the per-core limit.
"""

import sys

sys.path.insert(0, "/opt/trn_rl_repo")

import numpy as np
import orjson
import ml_dtypes

import concourse.bass as bass
import concourse.mybir as mybir
import concourse.tile as tile
from concourse import masks
from concourse.bass_utils import run_bass_kernel_spmd

F32 = mybir.dt.float32
F32R = mybir.dt.float32r
BF16 = mybir.dt.bfloat16
AF = mybir.ActivationFunctionType
ALU = mybir.AluOpType
AX = mybir.AxisListType

B = 64          # batch
NWALL = 1024    # total words
NCORES = 8
NW = NWALL // NCORES  # 128 words per core
D = 1024        # embed dim
C = 64          # word classes
NB = 32         # bin features
DIN = D + C + NB  # 1120
GRP = 4         # batches per matmul group (4 * 128 words = 512 tokens)
NGRP = B // GRP
NCH = 16        # batches per chain chunk

# groups whose features arrive via the SWDGE cast-load path as bf16;
# spaced ~2 groups apart to match the ~14us/2MB cast-path rate, first
# one late enough that its data can land before the group runs
SWG = frozenset({3, 6, 8, 10, 12, 14})


def _split_multiwait_json(raw: bytes) -> bytes:
    """walrus in this container only accepts 1 sync-wait per most
    instructions; Tile's final drain (and some others) carry several.
    Move extras onto preceding EventSemaphore carriers (2 waits each) on
    the same engine."""
    bir = orjson.loads(raw)
    for fn in bir["functions"]:
        for blk in fn["blocks"]:
            out = []
            for ins in blk["instructions"]:
                si = ins.get("sync_info")
                waits = (si or {}).get("on_wait") or []
                if len(waits) > 1:
                    extra = waits[:-1]
                    for k in range(0, len(extra), 2):
                        out.append(
                            {
                                "debug": ins.get("debug", 0),
                                "engine": ins["engine"],
                                "ins": [],
                                "outs": [],
                                "name": f"{ins['name']}_sw{k}",
                                "opcode": "EventSemaphore",
                                "sync_info": {
                                    "on_update": [],
                                    "on_wait": extra[k : k + 2],
                                },
                            }
                        )
                    si["on_wait"] = [waits[-1]]
                out.append(ins)
            blk["instructions"] = out
    return orjson.dumps(bir)


def build_program():
    nc = bass.Bass("TRN2", target_bir_lowering=False, debug=False)

    feat = nc.dram_tensor("feat", [B, NW, D], F32R, kind="ExternalInput")
    featb = nc.dram_tensor("featb", [B, NW, D], F32, kind="ExternalInput")
    wc = nc.dram_tensor("wc", [B, NW, C], F32, kind="ExternalInput")
    wtrd = nc.dram_tensor("wtr", [128, 9, NB], BF16, kind="ExternalInput")
    vrd = nc.dram_tensor("vrows", [128, NB, NB], BF16, kind="ExternalInput")
    bt = nc.dram_tensor("b", [NB, 128], F32, kind="ExternalInput")
    identd = nc.dram_tensor("ident", [128, 128], F32R, kind="ExternalInput")
    # out stays word-major ([w, b, i], matching Z's layout) so stores are
    # contiguous runs at line rate; the host transposes axes 0/1 after.
    out = nc.dram_tensor("out", [NW, B, NB], BF16, kind="ExternalOutput")

    with tile.TileContext(nc) as tc:
        with (
            tc.tile_pool(name="const", bufs=1) as constp,
            tc.tile_pool(name="x2", bufs=5) as x2p,
            tc.tile_pool(name="x2b", bufs=2) as x2bp,
            tc.tile_pool(name="xt", bufs=2) as xtp,
            tc.tile_pool(name="blt", bufs=2) as bltp,
            tc.tile_pool(name="tp", bufs=2, space="PSUM") as tpp,
            tc.tile_pool(name="wcps", bufs=1, space="PSUM") as wcpsp,
            tc.tile_pool(name="mmps", bufs=2, space="PSUM") as mmpsp,
            tc.tile_pool(name="petps", bufs=1, space="PSUM") as petpsp,
        ):
            # f32r identity from host (gpsimd memset can't touch f32r)
            identr = constp.tile([128, 128], F32R)
            nc.scalar.dma_start(identr[:], identd.ap())

            # wc halves early on both rings (softmax gates group 0's
            # final matmul); feature group halves follow.
            wcs = constp.tile([128, B, C], F32)
            wc_r = wc.ap().rearrange("b p c -> p b c")
            nc.sync.dma_start(wcs[0:64], wc_r[0:64])

            ident = constp.tile([128, 128], BF16)
            masks.make_identity(nc, ident[:])
            identf = constp.tile([128, 128], F32)
            masks.make_identity(nc, identf[:])

            # group-0/1 feature halves ahead of the weight tables
            x2_tiles = []
            for g in range(2):
                b0 = g * GRP
                x2 = x2p.tile([128, GRP, D], F32R, tag="x2")
                fr = feat.ap()[b0 : b0 + GRP, :, :].rearrange("b p d -> p b d")
                nc.sync.dma_start(x2[:, :, 0 : D // 2], fr[:, :, 0 : D // 2])
                nc.scalar.dma_start(x2[:, :, D // 2 : D], fr[:, :, D // 2 : D])
                x2_tiles.append(x2)
            nc.scalar.dma_start(wcs[64:128], wc_r[64:128])

            b_sb = constp.tile([NB, 128], F32)
            nc.scalar.dma_start(b_sb[:], bt.ap())
            wtr = constp.tile([128, 9, NB], BF16)
            nc.scalar.dma_start(wtr[:], wtrd.ap())
            vr = constp.tile([128, NB, NB], BF16)
            nc.scalar.dma_start(vr[:], vrd.ap())

            wcn = constp.tile([128, B, C], BF16)
            # token-major chain state: [words, batch, bins]; slot i holds
            # L_i until bin i's sigmoid overwrites it with p_i
            Z = constp.tile([128, B, NB], BF16)
            tmpA = constp.tile([128, NCH, NB + 1], BF16)
            zcA = constp.tile([128, NCH], F32)
            tmpB = constp.tile([128, NCH, NB + 1], BF16)
            zcB = constp.tile([128, NCH], F32)
            tmpC = constp.tile([128, NCH, NB + 1], BF16)
            zcC = constp.tile([128, NCH], F32)
            tmpD = constp.tile([128, NCH, NB + 1], BF16)
            zcD = constp.tile([128, NCH], F32)

            # ---------------- softmax over batch ----------------
            with tc.tile_pool(name="soft", bufs=1) as softp:
                ex = softp.tile([128, B, C], F32)
                nc.scalar.activation(ex[:], wcs[:], AF.Exp)
                acc = softp.tile([128, B // 2, C], F32)
                nc.vector.tensor_add(
                    acc[:], ex[:, 0 : B // 2, :], ex[:, B // 2 : B, :]
                )
                h = B // 4
                while h >= 1:
                    nc.vector.tensor_add(
                        acc[:, 0:h, :], acc[:, 0:h, :], acc[:, h : 2 * h, :]
                    )
                    h //= 2
                rec = softp.tile([128, C], F32)
                nc.vector.reciprocal(rec[:], acc[:, 0, :])
                nc.vector.tensor_mul(
                    wcn[:],
                    ex[:],
                    rec[:].unsqueeze(1).broadcast_to([128, B, C]),
                )

            # ---------------- chain helper ----------------
            # step = mul (GpSimd) -> reduce (DVE) -> sigmoid (ACT): one
            # op per engine so three chunks can be in flight at once.
            def chain_bin(i, bs, tmp, zc, mul_eng=None):
                nbt = bs.stop - bs.start
                if i == 0:
                    nc.scalar.activation(Z[:, bs, 0], Z[:, bs, 0], AF.Sigmoid)
                    return
                (mul_eng or nc.gpsimd).tensor_mul(
                    tmp[:, :, 0 : i + 1],
                    Z[:, bs, 0 : i + 1],
                    vr[:, i, 0 : i + 1]
                    .unsqueeze(1)
                    .broadcast_to([128, nbt, i + 1]),
                )
                nc.vector.reduce_sum(zc[:, :], tmp[:, :, 0 : i + 1], axis=AX.X)
                nc.scalar.activation(Z[:, bs, i], zc[:, :], AF.Sigmoid)

            def store_chunk(b0, b1):
                nc.sync.dma_start(out.ap()[:, b0:b1, :], Z[:, b0:b1, :])

            bsA = slice(0, NCH)
            bsB = slice(NCH, 2 * NCH)
            bsC = slice(2 * NCH, 3 * NCH)
            bsD = slice(3 * NCH, B)

            # per-group chain-step schedule: A bins 0-31 over groups
            # 4-11, B over 8-15, C bins 0-15 over 12-15.
            sched = {g: [] for g in range(NGRP)}
            for i in range(NB):
                sched[4 + i // 4].append(
                    lambda i=i: chain_bin(i, bsA, tmpA, zcA)
                )
            for i in range(NB):
                sched[8 + i // 4].append(
                    lambda i=i: chain_bin(i, bsB, tmpB, zcB)
                )
            for i in range(NCH):
                sched[12 + i // 4].append(
                    lambda i=i: chain_bin(i, bsC, tmpC, zcC)
                )
            # interleave so consecutive emissions never belong to the
            # same serial chain
            for g in range(NGRP):
                items = sched[g]
                if len(items) == 8:
                    sched[g] = [
                        items[j // 2] if j % 2 == 0 else items[4 + j // 2]
                        for j in range(8)
                    ]

            # ---------------- main matmul pipeline ----------------
            for g in range(NGRP):
                b0 = g * GRP
                todo = list(sched[g])
                per = max(1, (len(todo) + 5) // 6)

                def emit():
                    for _ in range(per):
                        if todo:
                            todo.pop(0)()

                if g < 2:
                    x2 = x2_tiles[g]
                    xdt, idop = F32R, identr
                elif g in SWG:
                    # SWDGE cast-load: lands as bf16, transposes at 1
                    # PE cycle/row, evacuates at 2x DVE rate
                    x2 = x2bp.tile([128, GRP, D], BF16, tag="x2b")
                    fr = featb.ap()[b0 : b0 + GRP, :, :].rearrange(
                        "b p d -> p b d"
                    )
                    nc.gpsimd.dma_start(x2[:], fr)
                    xdt, idop = BF16, ident
                else:
                    x2 = x2p.tile([128, GRP, D], F32R, tag="x2")
                    fr = feat.ap()[b0 : b0 + GRP, :, :].rearrange(
                        "b p d -> p b d"
                    )
                    nc.sync.dma_start(x2[:, :, 0 : D // 2], fr[:, :, 0 : D // 2])
                    nc.scalar.dma_start(x2[:, :, D // 2 : D], fr[:, :, D // 2 : D])
                    xdt, idop = F32R, identr
                xts = xtp.tile([128, 9, 512], BF16, tag="xt")
                for kh in range(4):
                    pt = tpp.tile([128, 2, 512], xdt, tag="xtps")
                    for kk in range(2):
                        k = kh * 2 + kk
                        for bi in range(GRP):
                            nc.tensor.transpose(
                                pt[:, kk, bi * 128 : (bi + 1) * 128],
                                x2[:, bi, k * 128 : (k + 1) * 128],
                                idop[:],
                            )
                    # cast-evacuation psum -> bf16 SBUF, split ACT/DVE
                    if kh % 2 == 0:
                        nc.scalar.copy(xts[:, kh * 2 : kh * 2 + 2, :], pt[:])
                    else:
                        nc.vector.tensor_copy(xts[:, kh * 2 : kh * 2 + 2, :], pt[:])
                    if kh < 2:
                        emit()

                # softmaxed wc as 9th k-chunk: transpose on chip
                wps = wcpsp.tile([64, 512], BF16, tag="wct")
                for bi in range(GRP):
                    nc.tensor.transpose(
                        wps[:, bi * 128 : (bi + 1) * 128],
                        wcn[:, b0 + bi, :],
                        ident[:],
                    )
                nc.scalar.copy(xts[0:64, 8, :], wps[:])
                emit()
                emit()

                ps = mmpsp.tile([NB, 512], F32, tag="mm")
                for k in range(8):
                    nc.tensor.matmul(
                        ps[:], wtr[:, k, :], xts[:, k, :],
                        start=(k == 0), stop=False,
                    )
                nc.tensor.matmul(
                    ps[:], wtr[0:64, 8, :], xts[0:64, 8, :],
                    start=False, stop=True,
                )
                blt = bltp.tile([NB, 512], F32, tag="blt")
                nc.scalar.activation(
                    blt[:], ps[:], AF.Identity, bias=b_sb[:, 0:1], scale=1.0
                )
                # corner turn: 4 x [32,128] -> one [128, 4*32] psum, one copy
                ptc = petpsp.tile([128, 128], F32, tag="pet")
                for q in range(GRP):
                    nc.tensor.transpose(
                        ptc[:, q * NB : (q + 1) * NB],
                        blt[:, q * 128 : (q + 1) * 128],
                        identf[0:NB, 0:NB],
                    )
                nc.vector.tensor_copy(Z[:, b0 : b0 + GRP, :], ptc[:])
                emit()
                emit()
                while todo:
                    todo.pop(0)()

                if g == 11:
                    store_chunk(0, NCH)      # A finished during g11
            store_chunk(NCH, 2 * NCH)        # B finished during g15

            # ---------------- tail: C resumes at 16, D runs 0-31 ------
            # two interleaved chains; D's muls on gpsimd, C's on DVE so
            # neither engine sees two ops of the same serial chain
            # back-to-back.
            ci, di = NCH, 0
            while ci < NB or di < NB:
                if di < NB:
                    chain_bin(di, bsD, tmpD, zcD, mul_eng=nc.gpsimd)
                    di += 1
                if ci < NB and (di % 2 == 0 or di >= NB):
                    chain_bin(ci, bsC, tmpC, zcC, mul_eng=nc.vector)
                    ci += 1
            store_chunk(2 * NCH, 3 * NCH)
            store_chunk(3 * NCH, B)

    orig = nc.to_json_bytes
    nc.to_json_bytes = lambda: _split_multiwait_json(orig())
    return nc


_PROG = None


def _get_prog():
    global _PROG
    if _PROG is None:
        _PROG = build_program()
    return _PROG


def _host_weights(W, b):
    """Host-side prep of the tiny weight tensors."""
    W = np.asarray(W, dtype=np.float32)
    wtr = np.zeros((128, 9, NB), dtype=ml_dtypes.bfloat16)
    for k in range(8):
        wtr[:, k, :] = W[:, k * 128 : (k + 1) * 128].T.astype(ml_dtypes.bfloat16)
    wtr[0:64, 8, :] = W[:, D : D + C].T.astype(ml_dtypes.bfloat16)
    wbin = W[:, D + C : DIN]  # [32, 32]
    vr = np.zeros((NB, NB), dtype=np.float32)
    for i in range(NB):
        vr[i, :i] = wbin[i, :i]
        vr[i, i] = 1.0
    vrows = np.broadcast_to(
        vr.astype(ml_dtypes.bfloat16)[None], (128, NB, NB)
    ).copy()
    bt = np.ascontiguousarray(
        np.tile(np.asarray(b, dtype=np.float32)[:, None], (1, 128))
    )
    return wtr, vrows, bt


def kernel(features, word_class_features, W, b, trace=False, tmpdir=None):
    features = np.ascontiguousarray(features, dtype=np.float32)
    word_class_features = np.ascontiguousarray(word_class_features, dtype=np.float32)
    wtr, vrows, bf = _host_weights(W, b)

    nc = _get_prog()
    in_maps = []
    for c in range(NCORES):
        sl = slice(c * NW, (c + 1) * NW)
        fshard = np.ascontiguousarray(features[:, sl, :])
        in_maps.append(
            {
                "feat": fshard,
                "featb": fshard,
                "wc": np.ascontiguousarray(word_class_features[:, sl, :]),
                "wtr": wtr,
                "vrows": vrows,
                "b": bf,
                "ident": np.eye(128, dtype=np.float32),
            }
        )
    res = run_bass_kernel_spmd(
        nc, in_maps, core_ids=list(range(NCORES)), trace=trace, tmpdir=tmpdir
    )
    # per-core out is word-major [NW, B, NB]; transpose to [B, NW, NB]
    outp = np.concatenate(
        [
            res.results[c]["out"].astype(np.float32).transpose(1, 0, 2)
            for c in range(NCORES)
        ],
        axis=1,
    )
    kernel._last_result = res
    return outp


# revision 14
# speedup vs baseline: 1.0305x; 1.0305x over previous
"""Bass/Tile kernel for nn_BinaryClassifierChain on 8 trn2 cores (v8).

Math (per reference.py):
  wc   = softmax(word_class_features, axis=0)            # over batch dim
  base = concat([features, wc], -1)                      # [B, W, 1088]
  L    = base @ W[:, :1088].T + b                        # [B, W, 32]
  chain: p_i = sigmoid(L_i + sum_{j<i} Wbin[i, j] p_j)   # Wbin = W[:, 1088:]

Sharding: pure data-parallel over the words dim (1024 = 8 x 128); the
batch-softmax stays intact per shard.

v8 vs v5c (v6/v7 experiments showed: gpsimd is bad for latency-critical
chain ops - ~0.5us semaphore waits; SWDGE cast prefetch starves the
startup loads; chain ops cost ~0.4-0.7us each regardless of payload):
  - software-pipelined PE emission: group g's transposes are emitted
    BEFORE group g-1's main matmuls, so the PE never idles waiting for
    an evacuation - by the time g's transposes drain, g-1's xts is
    long ready.  This also hides the softmax latency for group 0.
  - chain chunks of 16: A bins 0-31 at 4/group over g4-11, B over
    g8-15, C bins 0-15 over g12-15; tail is only C 16-31 + D 0-31 (48
    steps, two interleaved chains) instead of 80 steps.
  - load order: group-0 feature halves lead both rings, wc follows, so
    the first transposes start ~5us earlier.
"""

import sys

sys.path.insert(0, "/opt/trn_rl_repo")

import numpy as np
import orjson
import ml_dtypes

import concourse.bass as bass
import concourse.mybir as mybir
import concourse.tile as tile
from concourse import masks
from concourse.bass_utils import run_bass_kernel_spmd

F32 = mybir.dt.float32
F32R = mybir.dt.float32r
BF16 = mybir.dt.bfloat16
AF = mybir.ActivationFunctionType
ALU = mybir.AluOpType
AX = mybir.AxisListType

B = 64          # batch
NWALL = 1024    # total words
NCORES = 8
NW = NWALL // NCORES  # 128 words per core
D = 1024        # embed dim
C = 64          # word classes
NB = 32         # bin features
DIN = D + C + NB  # 1120
GRP = 4         # batches per matmul group (4 * 128 words = 512 tokens)
NGRP = B // GRP
NCH = 16        # batches per chain chunk


def _split_multiwait_json(raw: bytes) -> bytes:
    """walrus in this container only accepts 1 sync-wait per most
    instructions; Tile's final drain (and some others) carry several.
    Move extras onto preceding EventSemaphore carriers (2 waits each) on
    the same engine."""
    bir = orjson.loads(raw)
    for fn in bir["functions"]:
        for blk in fn["blocks"]:
            out = []
            for ins in blk["instructions"]:
                si = ins.get("sync_info")
                waits = (si or {}).get("on_wait") or []
                if len(waits) > 1:
                    extra = waits[:-1]
                    for k in range(0, len(extra), 2):
                        out.append(
                            {
                                "debug": ins.get("debug", 0),
                                "engine": ins["engine"],
                                "ins": [],
                                "outs": [],
                                "name": f"{ins['name']}_sw{k}",
                                "opcode": "EventSemaphore",
                                "sync_info": {
                                    "on_update": [],
                                    "on_wait": extra[k : k + 2],
                                },
                            }
                        )
                    si["on_wait"] = [waits[-1]]
                out.append(ins)
            blk["instructions"] = out
    return orjson.dumps(bir)


def build_program():
    nc = bass.Bass("TRN2", target_bir_lowering=False, debug=False)

    feat = nc.dram_tensor("feat", [B, NW, D], F32R, kind="ExternalInput")
    wc = nc.dram_tensor("wc", [B, NW, C], F32, kind="ExternalInput")
    wtrd = nc.dram_tensor("wtr", [128, 9, NB], BF16, kind="ExternalInput")
    vrd = nc.dram_tensor("vrows", [128, NB, NB], BF16, kind="ExternalInput")
    bt = nc.dram_tensor("b", [NB, 128], F32, kind="ExternalInput")
    identd = nc.dram_tensor("ident", [128, 128], F32R, kind="ExternalInput")
    # out stays word-major ([w, b, i], matching Z's layout) so stores are
    # contiguous runs at line rate; the host transposes axes 0/1 after.
    out = nc.dram_tensor("out", [NW, B, NB], BF16, kind="ExternalOutput")

    with tile.TileContext(nc) as tc:
        with (
            tc.tile_pool(name="const", bufs=1) as constp,
            tc.tile_pool(name="x2", bufs=6) as x2p,
            tc.tile_pool(name="xt", bufs=2) as xtp,
            tc.tile_pool(name="blt", bufs=2) as bltp,
            tc.tile_pool(name="tp", bufs=2, space="PSUM") as tpp,
            tc.tile_pool(name="wcps", bufs=1, space="PSUM") as wcpsp,
            tc.tile_pool(name="mmps", bufs=2, space="PSUM") as mmpsp,
            tc.tile_pool(name="petps", bufs=1, space="PSUM") as petpsp,
        ):
            # f32r identity from host (gpsimd memset can't touch f32r)
            identr = constp.tile([128, 128], F32R)
            nc.scalar.dma_start(identr[:], identd.ap())

            # group-0 halves lead both rings; wc follows immediately so
            # the softmax result is ready by the time group 0's final
            # matmul comes up (which is emitted after group 1's
            # transposes - see the software pipelining below).
            wcs = constp.tile([128, B, C], F32)
            wc_r = wc.ap().rearrange("b p c -> p b c")
            x2_tiles = []
            x2 = x2p.tile([128, GRP, D], F32R, tag="x2")
            fr = feat.ap()[0:GRP, :, :].rearrange("b p d -> p b d")
            nc.sync.dma_start(x2[:, :, 0 : D // 2], fr[:, :, 0 : D // 2])
            nc.scalar.dma_start(x2[:, :, D // 2 : D], fr[:, :, D // 2 : D])
            x2_tiles.append(x2)
            nc.sync.dma_start(wcs[0:64], wc_r[0:64])
            nc.scalar.dma_start(wcs[64:128], wc_r[64:128])

            ident = constp.tile([128, 128], BF16)
            masks.make_identity(nc, ident[:])
            identf = constp.tile([128, 128], F32)
            masks.make_identity(nc, identf[:])

            x2 = x2p.tile([128, GRP, D], F32R, tag="x2")
            fr = feat.ap()[GRP : 2 * GRP, :, :].rearrange("b p d -> p b d")
            nc.sync.dma_start(x2[:, :, 0 : D // 2], fr[:, :, 0 : D // 2])
            nc.scalar.dma_start(x2[:, :, D // 2 : D], fr[:, :, D // 2 : D])
            x2_tiles.append(x2)

            b_sb = constp.tile([NB, 128], F32)
            nc.scalar.dma_start(b_sb[:], bt.ap())
            wtr = constp.tile([128, 9, NB], BF16)
            nc.scalar.dma_start(wtr[:], wtrd.ap())
            vr = constp.tile([128, NB, NB], BF16)
            nc.scalar.dma_start(vr[:], vrd.ap())

            wcn = constp.tile([128, B, C], BF16)
            # token-major chain state: [words, batch, bins]; slot i holds
            # L_i until bin i's sigmoid overwrites it with p_i
            Z = constp.tile([128, B, NB], BF16)
            tmpA = constp.tile([128, NCH, NB + 1], BF16)
            zcA = constp.tile([128, NCH], F32)
            tmpB = constp.tile([128, NCH, NB + 1], BF16)
            zcB = constp.tile([128, NCH], F32)
            tmpC = constp.tile([128, NCH, NB + 1], BF16)
            zcC = constp.tile([128, NCH], F32)
            tmpD = constp.tile([128, NCH, NB + 1], BF16)
            zcD = constp.tile([128, NCH], F32)

            # ---------------- softmax over batch ----------------
            with tc.tile_pool(name="soft", bufs=1) as softp:
                ex = softp.tile([128, B, C], F32)
                nc.scalar.activation(ex[:], wcs[:], AF.Exp)
                acc = softp.tile([128, B // 2, C], F32)
                nc.vector.tensor_add(
                    acc[:], ex[:, 0 : B // 2, :], ex[:, B // 2 : B, :]
                )
                h = B // 4
                while h >= 1:
                    nc.vector.tensor_add(
                        acc[:, 0:h, :], acc[:, 0:h, :], acc[:, h : 2 * h, :]
                    )
                    h //= 2
                rec = softp.tile([128, C], F32)
                nc.vector.reciprocal(rec[:], acc[:, 0, :])
                nc.vector.tensor_mul(
                    wcn[:],
                    ex[:],
                    rec[:].unsqueeze(1).broadcast_to([128, B, C]),
                )

            # ---------------- chain helper ----------------
            def chain_bin(i, bs, tmp, zc):
                nbt = bs.stop - bs.start
                if i == 0:
                    nc.scalar.activation(Z[:, bs, 0], Z[:, bs, 0], AF.Sigmoid)
                    return
                nc.vector.tensor_mul(
                    tmp[:, :, 0 : i + 1],
                    Z[:, bs, 0 : i + 1],
                    vr[:, i, 0 : i + 1]
                    .unsqueeze(1)
                    .broadcast_to([128, nbt, i + 1]),
                )
                nc.vector.reduce_sum(zc[:, :], tmp[:, :, 0 : i + 1], axis=AX.X)
                nc.scalar.activation(Z[:, bs, i], zc[:, :], AF.Sigmoid)

            def store_chunk(b0, b1):
                nc.sync.dma_start(out.ap()[:, b0:b1, :], Z[:, b0:b1, :])

            bsA = slice(0, NCH)
            bsB = slice(NCH, 2 * NCH)
            bsC = slice(2 * NCH, 3 * NCH)
            bsD = slice(3 * NCH, B)

            # per-group chain-step schedule: A bins 0-31 over groups
            # 4-11, B over 8-15, C bins 0-15 over 12-15 (4/group each,
            # at most two chains active per group).
            sched = {g: [] for g in range(NGRP)}
            for i in range(NB):
                sched[4 + i // 4].append(
                    lambda i=i: chain_bin(i, bsA, tmpA, zcA)
                )
            for i in range(NB):
                sched[8 + i // 4].append(
                    lambda i=i: chain_bin(i, bsB, tmpB, zcB)
                )
            for i in range(NCH):
                sched[12 + i // 4].append(
                    lambda i=i: chain_bin(i, bsC, tmpC, zcC)
                )
            # interleave so consecutive emissions never belong to the
            # same serial chain
            for g in range(NGRP):
                items = sched[g]
                if len(items) == 8:
                    sched[g] = [
                        items[j // 2] if j % 2 == 0 else items[4 + j // 2]
                        for j in range(8)
                    ]

            # ---------------- main pipeline (PE software-pipelined) ----
            # iteration g emits: transposes(g), then mains(g-1).  The PE
            # queue therefore never waits on the current group's psum
            # evacuations before starting useful work.  Chain hooks at
            # iteration g pop from sched[g-1]: every logit a sched[g-1]
            # step reads was written by mains emitted in iteration g-1
            # or earlier, so emission order can never run ahead of data.
            pend = None  # (b0, xts) of the group awaiting its matmuls

            def mains(b0, xts, emit):
                ps = mmpsp.tile([NB, 512], F32, tag="mm")
                for k in range(8):
                    nc.tensor.matmul(
                        ps[:], wtr[:, k, :], xts[:, k, :],
                        start=(k == 0), stop=False,
                    )
                nc.tensor.matmul(
                    ps[:], wtr[0:64, 8, :], xts[0:64, 8, :],
                    start=False, stop=True,
                )
                emit()
                blt = bltp.tile([NB, 512], F32, tag="blt")
                nc.scalar.activation(
                    blt[:], ps[:], AF.Identity, bias=b_sb[:, 0:1], scale=1.0
                )
                # corner turn: 4 x [32,128] -> [128, 4*32] psum, one copy
                ptc = petpsp.tile([128, 128], F32, tag="pet")
                for q in range(GRP):
                    nc.tensor.transpose(
                        ptc[:, q * NB : (q + 1) * NB],
                        blt[:, q * 128 : (q + 1) * 128],
                        identf[0:NB, 0:NB],
                    )
                nc.vector.tensor_copy(Z[:, b0 : b0 + GRP, :], ptc[:])
                emit()

            todo = []

            def emit():
                for _ in range(emit.per):
                    if todo:
                        todo.pop(0)()

            emit.per = 1

            for g in range(NGRP):
                b0 = g * GRP
                # chain steps scheduled against the PREVIOUS group (its
                # logits are complete once iteration g-1's mains were
                # emitted)
                todo.extend(sched[g - 1] if g >= 1 else [])
                emit.per = max(1, (len(todo) + 5) // 6)

                if g < 2:
                    x2 = x2_tiles[g]
                else:
                    x2 = x2p.tile([128, GRP, D], F32R, tag="x2")
                    fr = feat.ap()[b0 : b0 + GRP, :, :].rearrange(
                        "b p d -> p b d"
                    )
                    nc.sync.dma_start(x2[:, :, 0 : D // 2], fr[:, :, 0 : D // 2])
                    nc.scalar.dma_start(x2[:, :, D // 2 : D], fr[:, :, D // 2 : D])
                xts = xtp.tile([128, 9, 512], BF16, tag="xt")
                for kh in range(4):
                    pt = tpp.tile([128, 2, 512], F32R, tag="xtps")
                    for kk in range(2):
                        k = kh * 2 + kk
                        for bi in range(GRP):
                            nc.tensor.transpose(
                                pt[:, kk, bi * 128 : (bi + 1) * 128],
                                x2[:, bi, k * 128 : (k + 1) * 128],
                                identr[:],
                            )
                    # cast-evacuation f32 psum -> bf16 SBUF, split ACT/DVE
                    if kh % 2 == 0:
                        nc.scalar.copy(xts[:, kh * 2 : kh * 2 + 2, :], pt[:])
                    else:
                        nc.vector.tensor_copy(xts[:, kh * 2 : kh * 2 + 2, :], pt[:])
                    if kh < 2:
                        emit()

                # softmaxed wc as 9th k-chunk: transpose on chip
                wps = wcpsp.tile([64, 512], BF16, tag="wct")
                for bi in range(GRP):
                    nc.tensor.transpose(
                        wps[:, bi * 128 : (bi + 1) * 128],
                        wcn[:, b0 + bi, :],
                        ident[:],
                    )
                nc.scalar.copy(xts[0:64, 8, :], wps[:])
                emit()

                if pend is not None:
                    mains(pend[0], pend[1], emit)
                pend = (b0, xts)
                while todo:
                    todo.pop(0)()

                if g == 13:
                    store_chunk(0, NCH)      # A drained during g12
            mains(pend[0], pend[1], lambda: None)
            for step in sched[NGRP - 1]:
                step()
            store_chunk(NCH, 2 * NCH)        # B finished

            # ---------------- tail: C resumes at 16, D runs 0-31 ------
            ci, di = NCH, 0
            while ci < NB or di < NB:
                if di < NB:
                    chain_bin(di, bsD, tmpD, zcD)
                    di += 1
                if ci < NB:
                    chain_bin(ci, bsC, tmpC, zcC)
                    ci += 1
            store_chunk(2 * NCH, 3 * NCH)
            store_chunk(3 * NCH, B)

    orig = nc.to_json_bytes
    nc.to_json_bytes = lambda: _split_multiwait_json(orig())
    return nc


_PROG = None


def _get_prog():
    global _PROG
    if _PROG is None:
        _PROG = build_program()
    return _PROG


def _host_weights(W, b):
    """Host-side prep of the tiny weight tensors."""
    W = np.asarray(W, dtype=np.float32)
    wtr = np.zeros((128, 9, NB), dtype=ml_dtypes.bfloat16)
    for k in range(8):
        wtr[:, k, :] = W[:, k * 128 : (k + 1) * 128].T.astype(ml_dtypes.bfloat16)
    wtr[0:64, 8, :] = W[:, D : D + C].T.astype(ml_dtypes.bfloat16)
    wbin = W[:, D + C : DIN]  # [32, 32]
    vr = np.zeros((NB, NB), dtype=np.float32)
    for i in range(NB):
        vr[i, :i] = wbin[i, :i]
        vr[i, i] = 1.0
    vrows = np.broadcast_to(
        vr.astype(ml_dtypes.bfloat16)[None], (128, NB, NB)
    ).copy()
    bt = np.ascontiguousarray(
        np.tile(np.asarray(b, dtype=np.float32)[:, None], (1, 128))
    )
    return wtr, vrows, bt


def kernel(features, word_class_features, W, b, trace=False, tmpdir=None):
    features = np.ascontiguousarray(features, dtype=np.float32)
    word_class_features = np.ascontiguousarray(word_class_features, dtype=np.float32)
    wtr, vrows, bf = _host_weights(W, b)

    nc = _get_prog()
    in_maps = []
    for c in range(NCORES):
        sl = slice(c * NW, (c + 1) * NW)
        in_maps.append(
            {
                "feat": np.ascontiguousarray(features[:, sl, :]),
                "wc": np.ascontiguousarray(word_class_features[:, sl, :]),
                "wtr": wtr,
                "vrows": vrows,
                "b": bf,
                "ident": np.eye(128, dtype=np.float32),
            }
        )
    res = run_bass_kernel_spmd(
        nc, in_maps, core_ids=list(range(NCORES)), trace=trace, tmpdir=tmpdir
    )
    # per-core out is word-major [NW, B, NB]; transpose to [B, NW, NB]
    outp = np.concatenate(
        [
            res.results[c]["out"].astype(np.float32).transpose(1, 0, 2)
            for c in range(NCORES)
        ],
        axis=1,
    )
    kernel._last_result = res
    return outp


# revision 15
# speedup vs baseline: 1.0352x; 1.0046x over previous
"""Bass/Tile kernel for nn_BinaryClassifierChain on 8 trn2 cores (v9).

Math (per reference.py):
  wc   = softmax(word_class_features, axis=0)            # over batch dim
  base = concat([features, wc], -1)                      # [B, W, 1088]
  L    = base @ W[:, :1088].T + b                        # [B, W, 32]
  chain: p_i = sigmoid(L_i + sum_{j<i} Wbin[i, j] p_j)   # Wbin = W[:, 1088:]

Sharding: pure data-parallel over the words dim (1024 = 8 x 128); the
batch-softmax stays intact per shard.

v9 = v5c (the best measured variant: split 1MB load halves per HWDGE
ring, f32r transposes, word-major out + per-chunk line-rate stores,
v4's proven chain schedule) plus ONLY a startup reorder: group-0's
feature halves lead both rings, wc's halves follow, then group 1, then
the weight tables.  v8b showed this pulls the first transposes from
~25us to ~15us; everything else in v8's restructure regressed, so the
v5c schedule is restored verbatim.
"""

import sys

sys.path.insert(0, "/opt/trn_rl_repo")

import numpy as np
import orjson
import ml_dtypes

import concourse.bass as bass
import concourse.mybir as mybir
import concourse.tile as tile
from concourse import masks
from concourse.bass_utils import run_bass_kernel_spmd

F32 = mybir.dt.float32
F32R = mybir.dt.float32r
BF16 = mybir.dt.bfloat16
AF = mybir.ActivationFunctionType
ALU = mybir.AluOpType
AX = mybir.AxisListType

B = 64          # batch
NWALL = 1024    # total words
NCORES = 8
NW = NWALL // NCORES  # 128 words per core
D = 1024        # embed dim
C = 64          # word classes
NB = 32         # bin features
DIN = D + C + NB  # 1120
GRP = 4         # batches per matmul group (4 * 128 words = 512 tokens)
NGRP = B // GRP

CH0 = 32        # chain chunk 0 = batches [0, CH0)


def _split_multiwait_json(raw: bytes) -> bytes:
    """walrus in this container only accepts 1 sync-wait per most
    instructions; Tile's final drain (and some others) carry several.
    Move extras onto preceding EventSemaphore carriers (2 waits each) on
    the same engine."""
    bir = orjson.loads(raw)
    for fn in bir["functions"]:
        for blk in fn["blocks"]:
            out = []
            for ins in blk["instructions"]:
                si = ins.get("sync_info")
                waits = (si or {}).get("on_wait") or []
                if len(waits) > 1:
                    extra = waits[:-1]
                    for k in range(0, len(extra), 2):
                        out.append(
                            {
                                "debug": ins.get("debug", 0),
                                "engine": ins["engine"],
                                "ins": [],
                                "outs": [],
                                "name": f"{ins['name']}_sw{k}",
                                "opcode": "EventSemaphore",
                                "sync_info": {
                                    "on_update": [],
                                    "on_wait": extra[k : k + 2],
                                },
                            }
                        )
                    si["on_wait"] = [waits[-1]]
                out.append(ins)
            blk["instructions"] = out
    return orjson.dumps(bir)


def build_program():
    nc = bass.Bass("TRN2", target_bir_lowering=False, debug=False)

    feat = nc.dram_tensor("feat", [B, NW, D], F32R, kind="ExternalInput")
    wc = nc.dram_tensor("wc", [B, NW, C], F32, kind="ExternalInput")
    wtrd = nc.dram_tensor("wtr", [128, 9, NB], BF16, kind="ExternalInput")
    vrd = nc.dram_tensor("vrows", [128, NB, NB], BF16, kind="ExternalInput")
    bt = nc.dram_tensor("b", [NB, 128], F32, kind="ExternalInput")
    identd = nc.dram_tensor("ident", [128, 128], F32R, kind="ExternalInput")
    # out stays word-major ([w, b, i], matching Z's layout) so stores are
    # contiguous runs at line rate; the host transposes axes 0/1 after.
    out = nc.dram_tensor("out", [NW, B, NB], BF16, kind="ExternalOutput")

    with tile.TileContext(nc) as tc:
        with (
            tc.tile_pool(name="const", bufs=1) as constp,
            tc.tile_pool(name="x2", bufs=6) as x2p,
            tc.tile_pool(name="xt", bufs=2) as xtp,
            tc.tile_pool(name="blt", bufs=2) as bltp,
            tc.tile_pool(name="tp", bufs=2, space="PSUM") as tpp,
            tc.tile_pool(name="wcps", bufs=1, space="PSUM") as wcpsp,
            tc.tile_pool(name="mmps", bufs=2, space="PSUM") as mmpsp,
            tc.tile_pool(name="petps", bufs=1, space="PSUM") as petpsp,
        ):
            # f32r identity from host (gpsimd memset can't touch f32r)
            identr = constp.tile([128, 128], F32R)
            nc.scalar.dma_start(identr[:], identd.ap())

            # group-0 halves lead both rings, wc follows, then group 1,
            # then the small weight tables.
            wcs = constp.tile([128, B, C], F32)
            wc_r = wc.ap().rearrange("b p c -> p b c")
            x2_tiles = []
            x2 = x2p.tile([128, GRP, D], F32R, tag="x2")
            fr = feat.ap()[0:GRP, :, :].rearrange("b p d -> p b d")
            nc.sync.dma_start(x2[:, :, 0 : D // 2], fr[:, :, 0 : D // 2])
            nc.scalar.dma_start(x2[:, :, D // 2 : D], fr[:, :, D // 2 : D])
            x2_tiles.append(x2)
            nc.sync.dma_start(wcs[0:64], wc_r[0:64])
            nc.scalar.dma_start(wcs[64:128], wc_r[64:128])

            ident = constp.tile([128, 128], BF16)
            masks.make_identity(nc, ident[:])
            identf = constp.tile([128, 128], F32)
            masks.make_identity(nc, identf[:])

            x2 = x2p.tile([128, GRP, D], F32R, tag="x2")
            fr = feat.ap()[GRP : 2 * GRP, :, :].rearrange("b p d -> p b d")
            nc.sync.dma_start(x2[:, :, 0 : D // 2], fr[:, :, 0 : D // 2])
            nc.scalar.dma_start(x2[:, :, D // 2 : D], fr[:, :, D // 2 : D])
            x2_tiles.append(x2)

            b_sb = constp.tile([NB, 128], F32)
            nc.scalar.dma_start(b_sb[:], bt.ap())
            wtr = constp.tile([128, 9, NB], BF16)
            nc.scalar.dma_start(wtr[:], wtrd.ap())
            vr = constp.tile([128, NB, NB], BF16)
            nc.scalar.dma_start(vr[:], vrd.ap())

            wcn = constp.tile([128, B, C], BF16)
            # token-major chain state: [words, batch, bins]; slot i holds
            # L_i until bin i's sigmoid overwrites it with p_i
            Z = constp.tile([128, B, NB], BF16)
            tmp0 = constp.tile([128, CH0, NB + 1], BF16)
            zc0 = constp.tile([128, CH0], F32)
            BH = (B - CH0) // 2
            tmp1 = constp.tile([128, BH, NB + 1], BF16)
            zc1 = constp.tile([128, BH], F32)
            BQ = BH // 2
            tmp2 = constp.tile([128, BQ, NB + 1], BF16)
            zc2 = constp.tile([128, BQ], F32)
            tmp3 = constp.tile([128, BQ, NB + 1], BF16)
            zc3 = constp.tile([128, BQ], F32)

            # ---------------- softmax over batch ----------------
            with tc.tile_pool(name="soft", bufs=1) as softp:
                ex = softp.tile([128, B, C], F32)
                nc.scalar.activation(ex[:], wcs[:], AF.Exp)
                acc = softp.tile([128, B // 2, C], F32)
                nc.vector.tensor_add(
                    acc[:], ex[:, 0 : B // 2, :], ex[:, B // 2 : B, :]
                )
                h = B // 4
                while h >= 1:
                    nc.vector.tensor_add(
                        acc[:, 0:h, :], acc[:, 0:h, :], acc[:, h : 2 * h, :]
                    )
                    h //= 2
                rec = softp.tile([128, C], F32)
                nc.vector.reciprocal(rec[:], acc[:, 0, :])
                nc.vector.tensor_mul(
                    wcn[:],
                    ex[:],
                    rec[:].unsqueeze(1).broadcast_to([128, B, C]),
                )

            # ---------------- chain helper ----------------
            def chain_bin(i, bs, tmp, zc):
                nbt = bs.stop - bs.start
                if i == 0:
                    nc.scalar.activation(Z[:, bs, 0], Z[:, bs, 0], AF.Sigmoid)
                    return
                nc.vector.tensor_mul(
                    tmp[:, :, 0 : i + 1],
                    Z[:, bs, 0 : i + 1],
                    vr[:, i, 0 : i + 1]
                    .unsqueeze(1)
                    .broadcast_to([128, nbt, i + 1]),
                )
                nc.vector.reduce_sum(zc[:, :], tmp[:, :, 0 : i + 1], axis=AX.X)
                nc.scalar.activation(Z[:, bs, i], zc[:, :], AF.Sigmoid)

            def store_chunk(b0, b1):
                nc.sync.dma_start(out.ap()[:, b0:b1, :], Z[:, b0:b1, :])

            bs0 = slice(0, CH0)
            bsA = slice(CH0, CH0 + BH)
            CH_SLOT_G0 = 8   # chunk-0 bins spread over groups 8..15

            def c0_bins_for(g, pos):
                if g < CH_SLOT_G0:
                    return []
                base = (g - CH_SLOT_G0) * 4
                return [base + pos] if pos < 4 else []

            def c1a_early(g, pos):
                """first 16 bins of the c1a half-chain (batches 32-47,
                ready after group 11) run during groups 12-15."""
                if g < 12:
                    return
                base = (g - 12) * 4
                if pos == 4:
                    chain_bin(base, bsA, tmp1, zc1)
                    chain_bin(base + 1, bsA, tmp1, zc1)
                else:
                    chain_bin(base + 2, bsA, tmp1, zc1)
                    chain_bin(base + 3, bsA, tmp1, zc1)

            # ---------------- main matmul pipeline ----------------
            for g in range(NGRP):
                b0 = g * GRP
                if g < 2:
                    x2 = x2_tiles[g]
                else:
                    x2 = x2p.tile([128, GRP, D], F32R, tag="x2")
                    fr = feat.ap()[b0 : b0 + GRP, :, :].rearrange(
                        "b p d -> p b d"
                    )
                    nc.sync.dma_start(x2[:, :, 0 : D // 2], fr[:, :, 0 : D // 2])
                    nc.scalar.dma_start(x2[:, :, D // 2 : D], fr[:, :, D // 2 : D])
                xts = xtp.tile([128, 9, 512], BF16, tag="xt")
                for kh in range(4):
                    pt = tpp.tile([128, 2, 512], F32R, tag="xtps")
                    for kk in range(2):
                        k = kh * 2 + kk
                        for bi in range(GRP):
                            nc.tensor.transpose(
                                pt[:, kk, bi * 128 : (bi + 1) * 128],
                                x2[:, bi, k * 128 : (k + 1) * 128],
                                identr[:],
                            )
                    # cast-evacuation f32 psum -> bf16 SBUF, split ACT/DVE
                    if kh % 2 == 0:
                        nc.scalar.copy(xts[:, kh * 2 : kh * 2 + 2, :], pt[:])
                    else:
                        nc.vector.tensor_copy(xts[:, kh * 2 : kh * 2 + 2, :], pt[:])
                    if kh < 2:
                        for i in c0_bins_for(g, kh):
                            chain_bin(i, bs0, tmp0, zc0)

                # softmaxed wc as 9th k-chunk: transpose on chip
                wps = wcpsp.tile([64, 512], BF16, tag="wct")
                for bi in range(GRP):
                    nc.tensor.transpose(
                        wps[:, bi * 128 : (bi + 1) * 128],
                        wcn[:, b0 + bi, :],
                        ident[:],
                    )
                nc.scalar.copy(xts[0:64, 8, :], wps[:])
                for i in c0_bins_for(g, 2):
                    chain_bin(i, bs0, tmp0, zc0)
                c1a_early(g, 4)

                ps = mmpsp.tile([NB, 512], F32, tag="mm")
                for k in range(8):
                    nc.tensor.matmul(
                        ps[:], wtr[:, k, :], xts[:, k, :],
                        start=(k == 0), stop=False,
                    )
                nc.tensor.matmul(
                    ps[:], wtr[0:64, 8, :], xts[0:64, 8, :],
                    start=False, stop=True,
                )
                blt = bltp.tile([NB, 512], F32, tag="blt")
                nc.scalar.activation(
                    blt[:], ps[:], AF.Identity, bias=b_sb[:, 0:1], scale=1.0
                )
                # corner turn: 4 x [32,128] -> one [128, 4*32] psum, one copy
                ptc = petpsp.tile([128, 128], F32, tag="pet")
                for q in range(GRP):
                    nc.tensor.transpose(
                        ptc[:, q * NB : (q + 1) * NB],
                        blt[:, q * 128 : (q + 1) * 128],
                        identf[0:NB, 0:NB],
                    )
                nc.vector.tensor_copy(Z[:, b0 : b0 + GRP, :], ptc[:])
                for i in c0_bins_for(g, 3):
                    chain_bin(i, bs0, tmp0, zc0)
                c1a_early(g, 5)

            # chunk-0 (batches 0..CH0) finished during the pipeline
            store_chunk(0, CH0)

            # ---------------- tail: 3-way interleaved chains ----------
            # c1a (16 batches) resumes at bin 16 (0-15 ran during g12-15);
            # the last 16 batches run as TWO 8-batch chains b1/b2 so that
            # consecutive ops never belong to the same serial chain.
            bsB1 = slice(CH0 + BH, CH0 + BH + BQ)
            bsB2 = slice(CH0 + BH + BQ, B)
            for t in range(NB):
                chain_bin(t, bsB1, tmp2, zc2)
                if t % 2 == 0 and 16 + t // 2 < NB:
                    chain_bin(16 + t // 2, bsA, tmp1, zc1)
                chain_bin(t, bsB2, tmp3, zc3)
                if t % 2 == 1 and 16 + t // 2 == NB - 1:
                    # c1a finished: overlap its store with b1/b2 remainder
                    store_chunk(CH0, CH0 + BH)
            store_chunk(CH0 + BH, B)

    orig = nc.to_json_bytes
    nc.to_json_bytes = lambda: _split_multiwait_json(orig())
    return nc


_PROG = None


def _get_prog():
    global _PROG
    if _PROG is None:
        _PROG = build_program()
    return _PROG


def _host_weights(W, b):
    """Host-side prep of the tiny weight tensors."""
    W = np.asarray(W, dtype=np.float32)
    wtr = np.zeros((128, 9, NB), dtype=ml_dtypes.bfloat16)
    for k in range(8):
        wtr[:, k, :] = W[:, k * 128 : (k + 1) * 128].T.astype(ml_dtypes.bfloat16)
    wtr[0:64, 8, :] = W[:, D : D + C].T.astype(ml_dtypes.bfloat16)
    wbin = W[:, D + C : DIN]  # [32, 32]
    vr = np.zeros((NB, NB), dtype=np.float32)
    for i in range(NB):
        vr[i, :i] = wbin[i, :i]
        vr[i, i] = 1.0
    vrows = np.broadcast_to(
        vr.astype(ml_dtypes.bfloat16)[None], (128, NB, NB)
    ).copy()
    bt = np.ascontiguousarray(
        np.tile(np.asarray(b, dtype=np.float32)[:, None], (1, 128))
    )
    return wtr, vrows, bt


def kernel(features, word_class_features, W, b, trace=False, tmpdir=None):
    features = np.ascontiguousarray(features, dtype=np.float32)
    word_class_features = np.ascontiguousarray(word_class_features, dtype=np.float32)
    wtr, vrows, bf = _host_weights(W, b)

    nc = _get_prog()
    in_maps = []
    for c in range(NCORES):
        sl = slice(c * NW, (c + 1) * NW)
        in_maps.append(
            {
                "feat": np.ascontiguousarray(features[:, sl, :]),
                "wc": np.ascontiguousarray(word_class_features[:, sl, :]),
                "wtr": wtr,
                "vrows": vrows,
                "b": bf,
                "ident": np.eye(128, dtype=np.float32),
            }
        )
    res = run_bass_kernel_spmd(
        nc, in_maps, core_ids=list(range(NCORES)), trace=trace, tmpdir=tmpdir
    )
    # per-core out is word-major [NW, B, NB]; transpose to [B, NW, NB]
    outp = np.concatenate(
        [
            res.results[c]["out"].astype(np.float32).transpose(1, 0, 2)
            for c in range(NCORES)
        ],
        axis=1,
    )
    kernel._last_result = res
    return outp


# revision 21
# speedup vs baseline: 1.0584x; 1.0224x over previous
"""Bass/Tile kernel for nn_BinaryClassifierChain on 8 trn2 cores (v9).

Math (per reference.py):
  wc   = softmax(word_class_features, axis=0)            # over batch dim
  base = concat([features, wc], -1)                      # [B, W, 1088]
  L    = base @ W[:, :1088].T + b                        # [B, W, 32]
  chain: p_i = sigmoid(L_i + sum_{j<i} Wbin[i, j] p_j)   # Wbin = W[:, 1088:]

Sharding: pure data-parallel over the words dim (1024 = 8 x 128); the
batch-softmax stays intact per shard.

v10 = v5c (the best measured variant: split 1MB load halves per HWDGE
ring, f32r transposes, word-major out + per-chunk line-rate stores,
v4's proven chain schedule and load order) with ONE change: in the
3-way tail, chain B2's muls run on the otherwise-idle GpSimd engine.
The tail is DVE-throughput-bound (~58us of DVE work in a ~55us span),
and B2 has ~2us of per-step latency slack inside the round, so the
gpsimd detour (~1.2us mul incl sem) hides while DVE sheds ~10us.
"""

import sys

sys.path.insert(0, "/opt/trn_rl_repo")

import numpy as np
import orjson
import ml_dtypes

import concourse.bass as bass
import concourse.mybir as mybir
import concourse.tile as tile
from concourse import masks
from concourse.bass_utils import run_bass_kernel_spmd

F32 = mybir.dt.float32
F32R = mybir.dt.float32r
BF16 = mybir.dt.bfloat16
AF = mybir.ActivationFunctionType
ALU = mybir.AluOpType
AX = mybir.AxisListType

B = 64          # batch
NWALL = 1024    # total words
NCORES = 8
NW = NWALL // NCORES  # 128 words per core
D = 1024        # embed dim
C = 64          # word classes
NB = 32         # bin features
DIN = D + C + NB  # 1120
GRP = 4         # batches per matmul group (4 * 128 words = 512 tokens)
NGRP = B // GRP

CH0 = 32        # chain chunk 0 = batches [0, CH0)


def _split_multiwait_json(raw: bytes) -> bytes:
    """walrus in this container only accepts 1 sync-wait per most
    instructions; Tile's final drain (and some others) carry several.
    Move extras onto preceding EventSemaphore carriers (2 waits each) on
    the same engine."""
    bir = orjson.loads(raw)
    for fn in bir["functions"]:
        for blk in fn["blocks"]:
            out = []
            for ins in blk["instructions"]:
                si = ins.get("sync_info")
                waits = (si or {}).get("on_wait") or []
                if len(waits) > 1:
                    extra = waits[:-1]
                    for k in range(0, len(extra), 2):
                        out.append(
                            {
                                "debug": ins.get("debug", 0),
                                "engine": ins["engine"],
                                "ins": [],
                                "outs": [],
                                "name": f"{ins['name']}_sw{k}",
                                "opcode": "EventSemaphore",
                                "sync_info": {
                                    "on_update": [],
                                    "on_wait": extra[k : k + 2],
                                },
                            }
                        )
                    si["on_wait"] = [waits[-1]]
                out.append(ins)
            blk["instructions"] = out
    return orjson.dumps(bir)


def build_program():
    nc = bass.Bass("TRN2", target_bir_lowering=False, debug=False)

    feat = nc.dram_tensor("feat", [B, NW, D], F32R, kind="ExternalInput")
    wc = nc.dram_tensor("wc", [B, NW, C], F32, kind="ExternalInput")
    wtrd = nc.dram_tensor("wtr", [128, 9, NB], BF16, kind="ExternalInput")
    vrd = nc.dram_tensor("vrows", [128, NB, NB], BF16, kind="ExternalInput")
    bt = nc.dram_tensor("b", [NB, 128], F32, kind="ExternalInput")
    identd = nc.dram_tensor("ident", [128, 128], F32R, kind="ExternalInput")
    # out stays word-major ([w, b, i], matching Z's layout) so stores are
    # contiguous runs at line rate; the host transposes axes 0/1 after.
    out = nc.dram_tensor("out", [NW, B, NB], BF16, kind="ExternalOutput")

    with tile.TileContext(nc) as tc:
        with (
            tc.tile_pool(name="const", bufs=1) as constp,
            tc.tile_pool(name="x2", bufs=5) as x2p,
            tc.tile_pool(name="xt", bufs=2) as xtp,
            tc.tile_pool(name="blt", bufs=2) as bltp,
            tc.tile_pool(name="tp", bufs=2, space="PSUM") as tpp,
            tc.tile_pool(name="wcps", bufs=1, space="PSUM") as wcpsp,
            tc.tile_pool(name="mmps", bufs=2, space="PSUM") as mmpsp,
            tc.tile_pool(name="petps", bufs=1, space="PSUM") as petpsp,
        ):
            # f32r identity from host (gpsimd memset can't touch f32r)
            identr = constp.tile([128, 128], F32R)
            nc.scalar.dma_start(identr[:], identd.ap())

            # wc halves first (its softmax gates every group's final
            # matmul), then the small weight tables, then group halves.
            wcs = constp.tile([128, B, C], F32)
            wc_r = wc.ap().rearrange("b p c -> p b c")
            nc.sync.dma_start(wcs[0:64], wc_r[0:64])
            nc.scalar.dma_start(wcs[64:128], wc_r[64:128])

            ident = constp.tile([128, 128], BF16)
            masks.make_identity(nc, ident[:])
            identf = constp.tile([128, 128], F32)
            masks.make_identity(nc, identf[:])

            b_sb = constp.tile([NB, 128], F32)
            nc.scalar.dma_start(b_sb[:], bt.ap())
            wtr = constp.tile([128, 9, NB], BF16)
            nc.scalar.dma_start(wtr[:], wtrd.ap())
            vr = constp.tile([128, NB, NB], BF16)
            nc.scalar.dma_start(vr[:], vrd.ap())
            x2_tiles = []

            wcn = constp.tile([128, B, C], BF16)
            # token-major chain state: [words, batch, bins]; slot i holds
            # L_i until bin i's sigmoid overwrites it with p_i
            Z = constp.tile([128, B, NB], BF16)
            tmp0 = constp.tile([128, CH0, NB + 1], BF16)
            zc0 = constp.tile([128, CH0], F32)
            BH = (B - CH0) // 2
            tmp1 = constp.tile([128, BH, NB + 1], BF16)
            zc1 = constp.tile([128, BH], F32)
            BQ = BH // 2
            tmp2 = constp.tile([128, BQ, NB + 1], BF16)
            zc2 = constp.tile([128, BQ], F32)
            tmp3 = constp.tile([128, BQ, NB + 1], BF16)
            zc3 = constp.tile([128, BQ], F32)

            # ---------------- softmax over batch ----------------
            with tc.tile_pool(name="soft", bufs=1) as softp:
                ex = softp.tile([128, B, C], F32)
                nc.scalar.activation(ex[:], wcs[:], AF.Exp)
                acc = softp.tile([128, B // 2, C], F32)
                nc.vector.tensor_add(
                    acc[:], ex[:, 0 : B // 2, :], ex[:, B // 2 : B, :]
                )
                h = B // 4
                while h >= 1:
                    nc.vector.tensor_add(
                        acc[:, 0:h, :], acc[:, 0:h, :], acc[:, h : 2 * h, :]
                    )
                    h //= 2
                rec = softp.tile([128, C], F32)
                nc.vector.reciprocal(rec[:], acc[:, 0, :])
                nc.vector.tensor_mul(
                    wcn[:],
                    ex[:],
                    rec[:].unsqueeze(1).broadcast_to([128, B, C]),
                )

            # ---------------- chain helper ----------------
            def chain_bin(i, bs, tmp, zc, mul_eng=None):
                nbt = bs.stop - bs.start
                if i == 0:
                    nc.scalar.activation(Z[:, bs, 0], Z[:, bs, 0], AF.Sigmoid)
                    return
                (mul_eng or nc.vector).tensor_mul(
                    tmp[:, :, 0 : i + 1],
                    Z[:, bs, 0 : i + 1],
                    vr[:, i, 0 : i + 1]
                    .unsqueeze(1)
                    .broadcast_to([128, nbt, i + 1]),
                )
                nc.vector.reduce_sum(zc[:, :], tmp[:, :, 0 : i + 1], axis=AX.X)
                nc.scalar.activation(Z[:, bs, i], zc[:, :], AF.Sigmoid)

            def store_chunk(b0, b1):
                nc.sync.dma_start(out.ap()[:, b0:b1, :], Z[:, b0:b1, :])

            bs0 = slice(0, CH0)
            bsA = slice(CH0, CH0 + BH)
            CH_SLOT_G0 = 8   # chunk-0 bins spread over groups 8..15

            def c0_bins_for(g, pos):
                if g < CH_SLOT_G0:
                    return []
                base = (g - CH_SLOT_G0) * 4
                return [base + pos] if pos < 4 else []

            def c1a_early(g, pos):
                """first 16 bins of the c1a half-chain (batches 32-47,
                ready after group 11) run during groups 12-15."""
                if g < 12:
                    return
                base = (g - 12) * 4
                if pos == 4:
                    chain_bin(base, bsA, tmp1, zc1)
                    chain_bin(base + 1, bsA, tmp1, zc1)
                else:
                    chain_bin(base + 2, bsA, tmp1, zc1)
                    chain_bin(base + 3, bsA, tmp1, zc1)

            # ---------------- main matmul pipeline ----------------
            for g in range(NGRP):
                b0 = g * GRP
                x2 = x2p.tile([128, GRP, D], F32R, tag="x2")
                fr = feat.ap()[b0 : b0 + GRP, :, :].rearrange("b p d -> p b d")
                nc.sync.dma_start(x2[:, :, 0 : D // 2], fr[:, :, 0 : D // 2])
                nc.scalar.dma_start(x2[:, :, D // 2 : D], fr[:, :, D // 2 : D])
                xts = xtp.tile([128, 9, 512], BF16, tag="xt")
                for kh in range(4):
                    pt = tpp.tile([128, 2, 512], F32R, tag="xtps")
                    for kk in range(2):
                        k = kh * 2 + kk
                        for bi in range(GRP):
                            nc.tensor.transpose(
                                pt[:, kk, bi * 128 : (bi + 1) * 128],
                                x2[:, bi, k * 128 : (k + 1) * 128],
                                identr[:],
                            )
                    # cast-evacuation f32 psum -> bf16 SBUF, split ACT/DVE
                    if kh % 2 == 0:
                        nc.scalar.copy(xts[:, kh * 2 : kh * 2 + 2, :], pt[:])
                    else:
                        nc.vector.tensor_copy(xts[:, kh * 2 : kh * 2 + 2, :], pt[:])
                    if kh < 2:
                        for i in c0_bins_for(g, kh):
                            chain_bin(i, bs0, tmp0, zc0)

                # softmaxed wc as 9th k-chunk: transpose on chip
                wps = wcpsp.tile([64, 512], BF16, tag="wct")
                for bi in range(GRP):
                    nc.tensor.transpose(
                        wps[:, bi * 128 : (bi + 1) * 128],
                        wcn[:, b0 + bi, :],
                        ident[:],
                    )
                nc.scalar.copy(xts[0:64, 8, :], wps[:])
                for i in c0_bins_for(g, 2):
                    chain_bin(i, bs0, tmp0, zc0)
                c1a_early(g, 4)

                ps = mmpsp.tile([NB, 512], F32, tag="mm")
                for k in range(8):
                    nc.tensor.matmul(
                        ps[:], wtr[:, k, :], xts[:, k, :],
                        start=(k == 0), stop=False,
                    )
                nc.tensor.matmul(
                    ps[:], wtr[0:64, 8, :], xts[0:64, 8, :],
                    start=False, stop=True,
                )
                blt = bltp.tile([NB, 512], F32, tag="blt")
                nc.scalar.activation(
                    blt[:], ps[:], AF.Identity, bias=b_sb[:, 0:1], scale=1.0
                )
                # corner turn: 4 x [32,128] -> one [128, 4*32] psum, one copy
                ptc = petpsp.tile([128, 128], F32, tag="pet")
                for q in range(GRP):
                    nc.tensor.transpose(
                        ptc[:, q * NB : (q + 1) * NB],
                        blt[:, q * 128 : (q + 1) * 128],
                        identf[0:NB, 0:NB],
                    )
                nc.vector.tensor_copy(Z[:, b0 : b0 + GRP, :], ptc[:])
                for i in c0_bins_for(g, 3):
                    chain_bin(i, bs0, tmp0, zc0)
                c1a_early(g, 5)

            # chunk-0 (batches 0..CH0) finished during the pipeline
            store_chunk(0, CH0)

            # ---------------- tail: 3-way interleaved chains ----------
            # c1a (16 batches) resumes at bin 16 (0-15 ran during g12-15);
            # the last 16 batches run as TWO 8-batch chains b1/b2 so that
            # consecutive ops never belong to the same serial chain.
            bsB1 = slice(CH0 + BH, CH0 + BH + BQ)
            bsB2 = slice(CH0 + BH + BQ, B)
            # B2's muls go to the otherwise-idle GpSimd: the tail is
            # DVE-throughput-bound and B2 has ~2us of latency slack per
            # step inside the 3-way round.
            for t in range(NB):
                chain_bin(t, bsB1, tmp2, zc2)
                if t % 2 == 0 and 16 + t // 2 < NB:
                    chain_bin(16 + t // 2, bsA, tmp1, zc1)
                chain_bin(t, bsB2, tmp3, zc3, mul_eng=nc.gpsimd)
                if t % 2 == 1 and 16 + t // 2 == NB - 1:
                    # c1a finished: overlap its store with b1/b2 remainder
                    store_chunk(CH0, CH0 + BH)
            store_chunk(CH0 + BH, B)

    orig = nc.to_json_bytes
    nc.to_json_bytes = lambda: _split_multiwait_json(orig())
    return nc


_PROG = None


def _get_prog():
    global _PROG
    if _PROG is None:
        _PROG = build_program()
    return _PROG


def _host_weights(W, b):
    """Host-side prep of the tiny weight tensors."""
    W = np.asarray(W, dtype=np.float32)
    wtr = np.zeros((128, 9, NB), dtype=ml_dtypes.bfloat16)
    for k in range(8):
        wtr[:, k, :] = W[:, k * 128 : (k + 1) * 128].T.astype(ml_dtypes.bfloat16)
    wtr[0:64, 8, :] = W[:, D : D + C].T.astype(ml_dtypes.bfloat16)
    wbin = W[:, D + C : DIN]  # [32, 32]
    vr = np.zeros((NB, NB), dtype=np.float32)
    for i in range(NB):
        vr[i, :i] = wbin[i, :i]
        vr[i, i] = 1.0
    vrows = np.broadcast_to(
        vr.astype(ml_dtypes.bfloat16)[None], (128, NB, NB)
    ).copy()
    bt = np.ascontiguousarray(
        np.tile(np.asarray(b, dtype=np.float32)[:, None], (1, 128))
    )
    return wtr, vrows, bt


def kernel(features, word_class_features, W, b, trace=False, tmpdir=None):
    features = np.ascontiguousarray(features, dtype=np.float32)
    word_class_features = np.ascontiguousarray(word_class_features, dtype=np.float32)
    wtr, vrows, bf = _host_weights(W, b)

    nc = _get_prog()
    in_maps = []
    for c in range(NCORES):
        sl = slice(c * NW, (c + 1) * NW)
        in_maps.append(
            {
                "feat": np.ascontiguousarray(features[:, sl, :]),
                "wc": np.ascontiguousarray(word_class_features[:, sl, :]),
                "wtr": wtr,
                "vrows": vrows,
                "b": bf,
                "ident": np.eye(128, dtype=np.float32),
            }
        )
    res = run_bass_kernel_spmd(
        nc, in_maps, core_ids=list(range(NCORES)), trace=trace, tmpdir=tmpdir
    )
    # per-core out is word-major [NW, B, NB]; transpose to [B, NW, NB]
    outp = np.concatenate(
        [
            res.results[c]["out"].astype(np.float32).transpose(1, 0, 2)
            for c in range(NCORES)
        ],
        axis=1,
    )
    kernel._last_result = res
    return outp
